# revision 29
# baseline (speedup 1.0000x reference)
"""Trainium2 Bass kernel for nn_Decoder_22273700397282 (sparse_attention).

Math (per batch b):
    a = concat([h_state, x], -1)                      # (S, 3072)
    bias = h_state.sum(0) @ Ws + ba + bs              # (3072,)
    et = tanh(a @ Wa + bias)                          # (S, 3072)
    attn[s] = softmax_feat(et[s])  if mask[s] else uniform 1/3072
    out = a[trigger] * sum_s attn[s]                  # (3072,)

Implementation notes:
  - Data-parallel over batch: core c owns batches 4c..4c+3. No collectives.
  - Masked rows contribute exactly (1/3072) each (softmax of a constant row),
    so only unmasked rows are computed: rows are compacted on the host and the
    per-batch uniform term n_masked/3072 is added at the end.
  - tanh(z) in [-1,1] makes softmax stable without max-subtraction:
    attn = exp(t) / rowsum(exp(t)).
  - Main matmul in fp8 e4m3 DoubleRow (inputs scaled x16, tanh applies
    scale=1/256), or bf16 when MODE="bf16".  The per-batch bias row rides as
    an extra bf16 accumulation chunk with one-hot contraction rows (bf16
    hi+lo split keeps the large bias term at ~f32 accuracy).
  - Row-softmax sum comes free via the activation accum_out; the weighted
    column sum over rows is a PE matmul with lhsT = indicator * (1/rowsum),
    accumulated across row-tiles in a dedicated PSUM region; the indicator
    also encodes batch membership (M=4) and zeroes padding rows.
  - Wa stays resident in SBUF; PE paces the Wa DMA stream during a k-outer
    phase-1 on tile 0, and each tile's column-sum is deferred behind the next
    tile's matmuls so PE never waits on the softmax chain.
"""
import math
from contextlib import ExitStack

import numpy as np
import ml_dtypes

import concourse.bacc as bacc
import concourse.tile as tile
import concourse.mybir as mybir
from concourse import bass_utils

BF16 = mybir.dt.bfloat16
FP8 = mybir.dt.float8e4
F32 = mybir.dt.float32
AFT = mybir.ActivationFunctionType
BF = ml_dtypes.bfloat16
F8 = ml_dtypes.float8_e4m3   # TRN e4m3: max normal 240

B, S, IN = 32, 512, 1024
D = 3 * IN            # 3072 features / out size
KD = 2 * IN           # 2048 h_state features
NB = 4                # batches per core
NCORES = 8
NCH = D // 512        # 6 output chunks of 512

MODE = "fp8"          # "fp8" (DoubleRow) or "bf16"
SC = 16.0             # fp8 input scale; z arrives in PSUM x(SC*SC)

LAST_EXEC_NS = None
_PROG_CACHE = {}


def _build_program(T, mode):
    """Bass program for T row-tiles of 128 compacted rows per core."""
    fp8 = mode == "fp8"
    KCD = 12 if fp8 else 24          # data contraction chunks
    tanh_scale = 1.0 / (SC * SC) if fp8 else 1.0
    pm = mybir.MatmulPerfMode.DoubleRow if fp8 else None

    nc = bacc.Bacc("TRN2", target_bir_lowering=False, debug=False)
    if fp8:
        at_h = nc.dram_tensor("at", [T, 128, KCD, 2, 128], FP8,
                              kind="ExternalInput")
        wa_h = nc.dram_tensor("wa", [KCD, 128, 2, D], FP8,
                              kind="ExternalInput")
    else:
        at_h = nc.dram_tensor("at", [T, 128, KCD, 128], BF16,
                              kind="ExternalInput")
        wa_h = nc.dram_tensor("wa", [KCD, 128, D], BF16, kind="ExternalInput")
    atb_h = nc.dram_tensor("atb", [T, 128, 128], BF16, kind="ExternalInput")
    wab_h = nc.dram_tensor("wab", [128, D], BF16, kind="ExternalInput")
    ind_h = nc.dram_tensor("ind", [128, T * NB], BF16, kind="ExternalInput")
    trig_h = nc.dram_tensor("trig", [NB, D], F32, kind="ExternalInput")
    ub_h = nc.dram_tensor("ub", [2, NB], BF16, kind="ExternalInput")
    out_h = nc.dram_tensor("out", [NB, D], F32, kind="ExternalOutput")

    with tile.TileContext(nc) as tc:
        with (
            tc.tile_pool(name="wa_pool", bufs=1) as wa_pool,
            tc.tile_pool(name="at_pool", bufs=2) as at_pool,
            tc.tile_pool(name="small", bufs=2) as small,
            tc.tile_pool(name="epool", bufs=2) as epool,
        ):
            def at_tile():
                if fp8:
                    return at_pool.tile([128, KCD, 2, 128], FP8, tag="at",
                                        name="at_sb")
                return at_pool.tile([128, KCD, 128], BF16, tag="at",
                                    name="at_sb")

            def lhsT_of(at, c):
                return at[:, c]

            def rhs_of(c, ni):
                sl = slice(ni * 512, (ni + 1) * 512)
                if fp8:
                    return wa[:, c, :, sl]
                return wa[:, c, sl]

            # tile 0/1 lhsT + the Wa chunk stream.  dma_start issue costs
            # ~650ns on the issuing engine's queue, so spread the startup
            # DMAs across three otherwise-idle engines to get data flowing
            # ~2x sooner.
            if fp8:
                wa = wa_pool.tile([128, KCD, 2, D], FP8)
            else:
                wa = wa_pool.tile([128, KCD, D], BF16)
            if fp8:
                nc.sync.dma_start(wa[:, 0, :, 0:1024], wa_h[0][:, :, 0:1024])
                nc.sync.dma_start(wa[:, 0, :, 1024:], wa_h[0][:, :, 1024:])
            else:
                nc.sync.dma_start(wa[:, 0], wa_h[0])
            at0 = at_tile()
            half_c = KCD // 2
            nc.scalar.dma_start(at0[:, :half_c], at_h[0, :, :half_c])
            nc.scalar.dma_start(at0[:, half_c:], at_h[0, :, half_c:])
            for k in range(1, KCD):
                nc.sync.dma_start(wa[:, k], wa_h[k])
            atb0 = at_pool.tile([128, 128], BF16, tag="atb", name="atb_sb")
            nc.scalar.dma_start(atb0[:], atb_h[0])
            if T > 1:
                at1 = at_tile()
                nc.scalar.dma_start(at1[:], at_h[1])
                atb1 = at_pool.tile([128, 128], BF16, tag="atb",
                                    name="atb_sb")
                nc.scalar.dma_start(atb1[:], atb_h[1])
            wab = wa_pool.tile([128, D], BF16)
            nc.scalar.dma_start(wab[:], wab_h[:])
            ind_all = wa_pool.tile([128, T * NB], BF16)
            nc.gpsimd.dma_start(ind_all[:], ind_h[:])
            trig_sb = wa_pool.tile([NB, D], F32)
            nc.gpsimd.dma_start(trig_sb[:], trig_h[:])
            ub_sb = wa_pool.tile([2, NB], BF16)
            nc.gpsimd.dma_start(ub_sb[:], ub_h[:])
            ones2 = wa_pool.tile([2, 512], BF16)
            nc.gpsimd.memset(ones2[:], 1.0)

            def softmax_tail(t, rp):
                """row-sum -> 1/r -> batch-indicator lhsT for the column sum"""
                r = small.tile([128, 1], F32)
                nc.vector.tensor_reduce(
                    r[:], rp[:], mybir.AxisListType.X, mybir.AluOpType.add)
                rinv = small.tile([128, 1], F32)
                nc.vector.reciprocal(rinv[:], r[:])
                lhsT4 = small.tile([128, NB], BF16)
                nc.vector.tensor_scalar_mul(
                    lhsT4[:], ind_all[:, t * NB:(t + 1) * NB], rinv[:])
                return lhsT4

            def mm_seq(ps, at, atb, ni, first, last):
                """full contraction into psum slice ps: data chunks + bias"""
                for c in range(KCD):
                    nc.tensor.matmul(
                        ps, lhsT_of(at, c), rhs_of(c, ni),
                        start=(c == 0) and first, stop=False, perf_mode=pm)
                nc.tensor.matmul(
                    ps, atb[:], wab[:, ni * 512:(ni + 1) * 512],
                    start=False, stop=last)

            # PSUM plan (8 banks, pools released LIFO):
            #   phase 1:  main(2, reserved) + passB(3) + passA(3) = 8
            #   phase 2:  main(2) + acc(6) = 8
            es_main, es_b, es_a = ExitStack(), ExitStack(), ExitStack()
            psum_main = es_main.enter_context(
                tc.tile_pool(name="psum_main", bufs=2, space="PSUM"))
            pB = es_b.enter_context(
                tc.tile_pool(name="psum_p1b", bufs=1, space="PSUM"))
            pA = es_a.enter_context(
                tc.tile_pool(name="psum_p1a", bufs=1, space="PSUM"))

            # ---- phase 1: k-outer over the Wa chunk stream so PE paces with
            # the DMA: per chunk, 6 matmuls for tile 0 (pools pA+pB) and 2 for
            # tile 1 (the reserved psum_main slots) = 8 open PSUM groups.
            # ScalarE then drains tile 1's pairs FIRST so the main-pool slots
            # recycle for tile 1's remaining chunks; tile-0's pass-B softmax
            # is deferred into the middle of tile 1 to keep PE fed.
            et0 = epool.tile([128, D], BF16, tag="et")
            rp0 = small.tile([128, NCH], F32, tag="rp")

            def act_pair(ps, et, rp, ni):
                tt = small.tile([128, 512], BF16, tag="tt")
                nc.scalar.activation(tt[:], ps, AFT.Tanh, scale=tanh_scale)
                nc.scalar.activation(
                    et[:, ni * 512:(ni + 1) * 512], tt[:], AFT.Exp,
                    accum_out=rp[:, ni:ni + 1],
                )

            def p1_act(ps3, nis):
                for ni in nis:
                    j = ni % 3
                    act_pair(ps3[:, j * 512:(j + 1) * 512], et0, rp0, ni)

            ps3A = pA.tile([128, 3 * 512], F32)
            ps3B = pB.tile([128, 3 * 512], F32)
            if T > 1:
                et1 = epool.tile([128, D], BF16, tag="et")
                rp1 = small.tile([128, NCH], F32, tag="rp")
                ps_t1 = [psum_main.tile([128, 512], F32, name="ps")
                         for _ in range(2)]

            # the 8 bias matmuls depend only on wab — weave them into the
            # per-chunk arrival gaps of the Wa stream (one after each of
            # chunks 1..8) as PE filler
            def p1_bias(g):
                if g < 6:
                    ps3 = ps3A if g < 3 else ps3B
                    j = g % 3
                    nc.tensor.matmul(
                        ps3[:, j * 512:(j + 1) * 512],
                        atb0[:], wab[:, g * 512:(g + 1) * 512],
                        start=False, stop=False)
                elif T > 1:
                    ni = g - 6
                    nc.tensor.matmul(
                        ps_t1[ni][:], atb1[:],
                        wab[:, ni * 512:(ni + 1) * 512],
                        start=False, stop=False)

            for c in range(KCD):
                last_c = c == KCD - 1
                for half, ps3 in ((0, ps3A), (1, ps3B)):
                    for j in range(3):
                        ni = 3 * half + j
                        nc.tensor.matmul(
                            ps3[:, j * 512:(j + 1) * 512],
                            lhsT_of(at0, c), rhs_of(c, ni),
                            start=(c == 0), stop=last_c, perf_mode=pm)
                if T > 1:
                    for ni in range(2):
                        nc.tensor.matmul(
                            ps_t1[ni][:], lhsT_of(at1, c), rhs_of(c, ni),
                            start=(c == 0), stop=last_c, perf_mode=pm)
                if 1 <= c <= 8:
                    p1_bias(c - 1)
            def main_chunk(at, atb, et, rp, ni):
                ps = psum_main.tile([128, 512], F32, name="ps")
                mm_seq(ps[:], at, atb, ni, True, True)
                act_pair(ps[:], et, rp, ni)

            # Tile-0's six deferred softmax pairs are WOVEN between tile-1's
            # chunks on ScalarE: tile-1's pair must land in time to recycle
            # its PSUM slot, tile-0's pairs fill the gaps.
            if T > 1:
                for ni in range(2):
                    act_pair(ps_t1[ni][:], et1, rp1, ni)
                p1_act(ps3A, range(0, 1))
                main_chunk(at1, atb1, et1, rp1, 2)
                p1_act(ps3A, range(1, 2))
                main_chunk(at1, atb1, et1, rp1, 3)
                p1_act(ps3A, range(2, 3))
                es_a.close()
                main_chunk(at1, atb1, et1, rp1, 4)
                p1_act(ps3B, range(3, 4))
                main_chunk(at1, atb1, et1, rp1, 5)
                p1_act(ps3B, range(4, 6))
                es_b.close()
            else:
                p1_act(ps3A, range(0, 3))
                es_a.close()
                p1_act(ps3B, range(3, 6))
                es_b.close()

            # ---- phase 2: steady state; tile t-1's column-sum is emitted
            # after tile t's main matmuls so PE never waits on the softmax
            # reduction chain.
            with tc.tile_pool(name="psum_acc", bufs=1, space="PSUM") as psum_acc:
                psA = psum_acc.tile([NB, D], F32)

                def colsum(t, rp, et):
                    lhsT4 = softmax_tail(t, rp)
                    for ni in range(NCH):
                        nc.tensor.matmul(
                            psA[:, ni * 512:(ni + 1) * 512],
                            lhsT4[:],
                            et[:, ni * 512:(ni + 1) * 512],
                            start=(t == 0), stop=False,
                        )

                colsum(0, rp0, et0)
                prev = (1, rp1, et1) if T > 1 else None

                for t in range(2, T):
                    at = at_tile()
                    nc.sync.dma_start(at[:], at_h[t])
                    atb = at_pool.tile([128, 128], BF16, tag="atb",
                                       name="atb_sb")
                    nc.sync.dma_start(atb[:], atb_h[t])
                    et = epool.tile([128, D], BF16, tag="et")
                    rp = small.tile([128, NCH], F32, tag="rp")
                    if t < T - 1:
                        for ni in range(NCH):
                            main_chunk(at, atb, et, rp, ni)
                        colsum(*prev)
                        prev = (t, rp, et)
                    else:
                        # last tile: interleave the previous tile's column-sum
                        # and the +u matmuls between its chunks so PE has fill
                        # work while the final softmax chain resolves; the
                        # final column-sum then closes each psA group.
                        tp, rpp, etp = prev
                        lhsT4p = softmax_tail(tp, rpp)
                        for ni in range(NCH):
                            main_chunk(at, atb, et, rp, ni)
                            sl = slice(ni * 512, (ni + 1) * 512)
                            nc.tensor.matmul(
                                psA[:, sl], lhsT4p[:], etp[:, sl],
                                start=False, stop=False)
                        for ni in range(NCH):
                            sl = slice(ni * 512, (ni + 1) * 512)
                            nc.tensor.matmul(
                                psA[:, sl], ub_sb[:], ones2[:],
                                start=False, stop=False)
                        lhsT4 = softmax_tail(t, rp)
                        for ni in range(NCH):
                            sl = slice(ni * 512, (ni + 1) * 512)
                            nc.tensor.matmul(
                                psA[:, sl], lhsT4[:], et[:, sl],
                                start=False, stop=True)
                        prev = None
                u_done = T > 2
                if prev is not None:
                    # T == 2: plain final column-sum; +u closes the groups
                    colsum(*prev)
                for ni in range(NCH):
                    sl = slice(ni * 512, (ni + 1) * 512)
                    if not u_done:
                        nc.tensor.matmul(
                            psA[:, sl], ub_sb[:], ones2[:],
                            start=False, stop=True)
                    outn = small.tile([NB, 512], F32)
                    nc.vector.tensor_mul(outn[:], psA[:, sl], trig_sb[:, sl])
                    nc.sync.dma_start(out_h[:, sl], outn[:])
            es_main.close()
    nc.compile()
    return nc


def kernel(h_state, x, trigger, mask, Wa, ba, Ws, bs, *, trace=False):
    global LAST_EXEC_NS
    h_state = np.asarray(h_state, dtype=np.float32)
    x = np.asarray(x, dtype=np.float32)
    trigger = np.asarray(trigger).astype(np.int64)
    mask = np.asarray(mask)
    Wa = np.asarray(Wa, dtype=np.float32)
    ba = np.asarray(ba, dtype=np.float32)
    Ws = np.asarray(Ws, dtype=np.float32)
    bs = np.asarray(bs, dtype=np.float32)
    fp8 = MODE == "fp8"

    # per-batch bias row (f64 for accuracy; dominates z's magnitude)
    s_sum = h_state.sum(axis=1, dtype=np.float64)                  # (B, 2048)
    bias = (s_sum @ Ws.astype(np.float64)
            + ba.astype(np.float64) + bs.astype(np.float64)).astype(np.float32)
    # bias rides in a bf16 chunk with one-hot value ALPHA; its PSUM
    # contribution must come out x(SC*SC) in fp8 mode (tanh rescales).
    zscale = SC * SC if fp8 else 1.0
    alpha = SC if fp8 else 1.0
    beta = zscale / alpha
    bias_hi = (bias * beta).astype(BF)
    bias_lo = (bias * beta - bias_hi.astype(np.float32)).astype(BF)  # (B, D)

    # trigger rows of a = [h_state | x]
    bi = np.arange(B)
    trig_full = np.concatenate(
        [h_state[bi, trigger], x[bi, trigger]], axis=1)            # (B, D)

    keep = [np.flatnonzero(np.asarray(mask[b]) != 0) for b in range(B)]
    n_rows_core = [
        sum(len(keep[c * NB + j]) for j in range(NB)) for c in range(NCORES)]
    T = max(1, max(math.ceil(r / 128) for r in n_rows_core))

    # shared quantized weight block
    if fp8:
        waq = np.clip(Wa * SC, -240.0, 240.0).astype(F8)
        # wa[c, p, r, n] = Wa_q[c*256 + r*128 + p, n]
        wa_dev = np.ascontiguousarray(
            waq.reshape(12, 2, 128, D).transpose(0, 2, 1, 3))
    else:
        wa_dev = np.ascontiguousarray(Wa.astype(BF).reshape(24, 128, D))

    in_maps = []
    for c in range(NCORES):
        rows_h = []           # compacted h_state rows
        rows_x = []           # compacted x rows
        owner = []            # batch-within-core per row
        for j in range(NB):
            b = c * NB + j
            idx = keep[b]
            rows_h.append(h_state[b, idx])
            rows_x.append(x[b, idx])
            owner.append(np.full(len(idx), j, dtype=np.int64))
        rows_h = np.concatenate(rows_h, axis=0)
        rows_x = np.concatenate(rows_x, axis=0)
        owner = np.concatenate(owner, axis=0)
        rc = rows_h.shape[0]
        r_idx = np.arange(rc)

        a_c = np.zeros((T * 128, D), dtype=np.float32)
        a_c[:rc, :KD] = rows_h
        a_c[:rc, KD:D] = rows_x
        if fp8:
            a_q = np.clip(a_c * SC, -240.0, 240.0).astype(F8)
            # at[t, p, c, r, m] = a_q[t*128+m, c*256 + r*128 + p]
            att = np.ascontiguousarray(
                a_q.reshape(T, 128, 12, 2, 128).transpose(0, 4, 2, 3, 1))
        else:
            att = np.ascontiguousarray(
                a_c.astype(BF).reshape(T, 128, 24, 128).transpose(0, 3, 2, 1))

        # bias chunk lhsT: atb[t, p, m] = alpha at p = 2*owner(+1) of row m
        atb = np.zeros((T * 128, 128), dtype=np.float32)
        atb[r_idx, 2 * owner] = alpha
        atb[r_idx, 2 * owner + 1] = alpha
        atb = np.ascontiguousarray(
            atb.astype(BF).reshape(T, 128, 128).transpose(0, 2, 1))

        # bias chunk rhs: rows 2j / 2j+1 = hi/lo of batch j
        wab = np.zeros((128, D), dtype=BF)
        for j in range(NB):
            b = c * NB + j
            wab[2 * j] = bias_hi[b]
            wab[2 * j + 1] = bias_lo[b]

        ind_all = np.zeros((128, T * NB), dtype=BF)
        ind_all[r_idx % 128, (r_idx // 128) * NB + owner] = 1.0

        trig = np.ascontiguousarray(trig_full[c * NB:(c + 1) * NB])
        u = np.array(
            [(S - len(keep[c * NB + j])) / np.float32(D) for j in range(NB)],
            dtype=np.float32)
        u_hi = u.astype(BF)
        u_lo = (u - u_hi.astype(np.float32)).astype(BF)
        ub = np.stack([u_hi, u_lo])                              # (2, NB)
        in_maps.append({"at": att, "atb": atb, "wa": wa_dev, "wab": wab,
                        "ind": ind_all, "trig": trig, "ub": ub})

    key = (T, MODE)
    if key not in _PROG_CACHE:
        _PROG_CACHE[key] = _build_program(T, MODE)
    nc = _PROG_CACHE[key]

    res = bass_utils.run_bass_kernel_spmd(
        nc, in_maps, list(range(NCORES)), trace=trace)
    LAST_EXEC_NS = res.exec_time_ns
    return np.concatenate(
        [np.asarray(res.results[c]["out"]) for c in range(NCORES)], axis=0)


# revision 30
# speedup vs baseline: 1.0022x; 1.0022x over previous
"""Trainium2 Bass kernel for nn_Decoder_22273700397282 (sparse_attention).

Math (per batch b):
    a = concat([h_state, x], -1)                      # (S, 3072)
    bias = h_state.sum(0) @ Ws + ba + bs              # (3072,)
    et = tanh(a @ Wa + bias)                          # (S, 3072)
    attn[s] = softmax_feat(et[s])  if mask[s] else uniform 1/3072
    out = a[trigger] * sum_s attn[s]                  # (3072,)

Implementation notes:
  - Data-parallel over batch: core c owns batches 4c..4c+3. No collectives.
  - Masked rows contribute exactly (1/3072) each (softmax of a constant row),
    so only unmasked rows are computed: rows are compacted on the host and the
    per-batch uniform term n_masked/3072 is added at the end.
  - tanh(z) in [-1,1] makes softmax stable without max-subtraction:
    attn = exp(t) / rowsum(exp(t)).
  - Main matmul in fp8 e4m3 DoubleRow (inputs scaled x16, tanh applies
    scale=1/256), or bf16 when MODE="bf16".  The per-batch bias row rides as
    an extra bf16 accumulation chunk with one-hot contraction rows (bf16
    hi+lo split keeps the large bias term at ~f32 accuracy).
  - Row-softmax sum comes free via the activation accum_out; the weighted
    column sum over rows is a PE matmul with lhsT = indicator * (1/rowsum),
    accumulated across row-tiles in a dedicated PSUM region; the indicator
    also encodes batch membership (M=4) and zeroes padding rows.
  - Wa stays resident in SBUF; PE paces the Wa DMA stream during a k-outer
    phase-1 on tile 0, and each tile's column-sum is deferred behind the next
    tile's matmuls so PE never waits on the softmax chain.
"""
import math
from contextlib import ExitStack

import numpy as np
import ml_dtypes

import concourse.bacc as bacc
import concourse.tile as tile
import concourse.mybir as mybir
from concourse import bass_utils

BF16 = mybir.dt.bfloat16
FP8 = mybir.dt.float8e4
F32 = mybir.dt.float32
AFT = mybir.ActivationFunctionType
BF = ml_dtypes.bfloat16
F8 = ml_dtypes.float8_e4m3   # TRN e4m3: max normal 240

B, S, IN = 32, 512, 1024
D = 3 * IN            # 3072 features / out size
KD = 2 * IN           # 2048 h_state features
NB = 4                # batches per core
NCORES = 8
NCH = D // 512        # 6 output chunks of 512

MODE = "fp8"          # "fp8" (DoubleRow) or "bf16"
SC = 16.0             # fp8 input scale; z arrives in PSUM x(SC*SC)

LAST_EXEC_NS = None
_PROG_CACHE = {}


def _build_program(T, mode):
    """Bass program for T row-tiles of 128 compacted rows per core."""
    fp8 = mode == "fp8"
    KCD = 12 if fp8 else 24          # data contraction chunks
    tanh_scale = 1.0 / (SC * SC) if fp8 else 1.0
    pm = mybir.MatmulPerfMode.DoubleRow if fp8 else None

    nc = bacc.Bacc("TRN2", target_bir_lowering=False, debug=False)
    if fp8:
        at_h = nc.dram_tensor("at", [T, 128, KCD, 2, 128], FP8,
                              kind="ExternalInput")
        wa_h = nc.dram_tensor("wa", [KCD, 128, 2, D], FP8,
                              kind="ExternalInput")
    else:
        at_h = nc.dram_tensor("at", [T, 128, KCD, 128], BF16,
                              kind="ExternalInput")
        wa_h = nc.dram_tensor("wa", [KCD, 128, D], BF16, kind="ExternalInput")
    atb_h = nc.dram_tensor("atb", [T, 128, 128], BF16, kind="ExternalInput")
    wab_h = nc.dram_tensor("wab", [128, D], BF16, kind="ExternalInput")
    ind_h = nc.dram_tensor("ind", [128, T * NB], BF16, kind="ExternalInput")
    trig_h = nc.dram_tensor("trig", [NB, D], F32, kind="ExternalInput")
    ub_h = nc.dram_tensor("ub", [2, NB], BF16, kind="ExternalInput")
    out_h = nc.dram_tensor("out", [NB, D], F32, kind="ExternalOutput")

    with tile.TileContext(nc) as tc:
        with (
            tc.tile_pool(name="wa_pool", bufs=1) as wa_pool,
            tc.tile_pool(name="at_pool", bufs=2) as at_pool,
            tc.tile_pool(name="small", bufs=2) as small,
            tc.tile_pool(name="epool", bufs=2) as epool,
        ):
            def at_tile():
                if fp8:
                    return at_pool.tile([128, KCD, 2, 128], FP8, tag="at",
                                        name="at_sb")
                return at_pool.tile([128, KCD, 128], BF16, tag="at",
                                    name="at_sb")

            def lhsT_of(at, c):
                return at[:, c]

            def rhs_of(c, ni):
                sl = slice(ni * 512, (ni + 1) * 512)
                if fp8:
                    return wa[:, c, :, sl]
                return wa[:, c, sl]

            # tile 0/1 lhsT + the Wa chunk stream.  dma_start issue costs
            # ~650ns on the issuing engine's queue, so spread the startup
            # DMAs across three otherwise-idle engines to get data flowing
            # ~2x sooner.
            if fp8:
                wa = wa_pool.tile([128, KCD, 2, D], FP8)
            else:
                wa = wa_pool.tile([128, KCD, D], BF16)
            if fp8:
                nc.sync.dma_start(wa[:, 0, :, 0:1024], wa_h[0][:, :, 0:1024])
                nc.sync.dma_start(wa[:, 0, :, 1024:], wa_h[0][:, :, 1024:])
            else:
                nc.sync.dma_start(wa[:, 0], wa_h[0])
            at0 = at_tile()
            half_c = KCD // 2
            nc.scalar.dma_start(at0[:, :half_c], at_h[0, :, :half_c])
            nc.scalar.dma_start(at0[:, half_c:], at_h[0, :, half_c:])
            for k in range(1, KCD):
                nc.sync.dma_start(wa[:, k], wa_h[k])
            wab = wa_pool.tile([128, D], BF16)
            nc.scalar.dma_start(wab[:], wab_h[:])
            atb0 = at_pool.tile([128, 128], BF16, tag="atb", name="atb_sb")
            nc.scalar.dma_start(atb0[:], atb_h[0])
            if T > 1:
                at1 = at_tile()
                nc.scalar.dma_start(at1[:], at_h[1])
                atb1 = at_pool.tile([128, 128], BF16, tag="atb",
                                    name="atb_sb")
                nc.scalar.dma_start(atb1[:], atb_h[1])
            ind_all = wa_pool.tile([128, T * NB], BF16)
            nc.gpsimd.dma_start(ind_all[:], ind_h[:])
            trig_sb = wa_pool.tile([NB, D], F32)
            nc.gpsimd.dma_start(trig_sb[:], trig_h[:])
            ub_sb = wa_pool.tile([2, NB], BF16)
            nc.gpsimd.dma_start(ub_sb[:], ub_h[:])
            ones2 = wa_pool.tile([2, 512], BF16)
            nc.gpsimd.memset(ones2[:], 1.0)

            def softmax_tail(t, rp):
                """row-sum -> 1/r -> batch-indicator lhsT for the column sum"""
                r = small.tile([128, 1], F32)
                nc.vector.tensor_reduce(
                    r[:], rp[:], mybir.AxisListType.X, mybir.AluOpType.add)
                rinv = small.tile([128, 1], F32)
                nc.vector.reciprocal(rinv[:], r[:])
                lhsT4 = small.tile([128, NB], BF16)
                nc.vector.tensor_scalar_mul(
                    lhsT4[:], ind_all[:, t * NB:(t + 1) * NB], rinv[:])
                return lhsT4

            def mm_seq(ps, at, atb, ni, first, last):
                """full contraction into psum slice ps: data chunks + bias"""
                for c in range(KCD):
                    nc.tensor.matmul(
                        ps, lhsT_of(at, c), rhs_of(c, ni),
                        start=(c == 0) and first, stop=False, perf_mode=pm)
                nc.tensor.matmul(
                    ps, atb[:], wab[:, ni * 512:(ni + 1) * 512],
                    start=False, stop=last)

            # PSUM plan (8 banks, pools released LIFO):
            #   phase 1:  main(2, reserved) + passB(3) + passA(3) = 8
            #   phase 2:  main(2) + acc(6) = 8
            es_main, es_b, es_a = ExitStack(), ExitStack(), ExitStack()
            psum_main = es_main.enter_context(
                tc.tile_pool(name="psum_main", bufs=2, space="PSUM"))
            pB = es_b.enter_context(
                tc.tile_pool(name="psum_p1b", bufs=1, space="PSUM"))
            pA = es_a.enter_context(
                tc.tile_pool(name="psum_p1a", bufs=1, space="PSUM"))

            # ---- phase 1: k-outer over the Wa chunk stream so PE paces with
            # the DMA: per chunk, 6 matmuls for tile 0 (pools pA+pB) and 2 for
            # tile 1 (the reserved psum_main slots) = 8 open PSUM groups.
            # ScalarE then drains tile 1's pairs FIRST so the main-pool slots
            # recycle for tile 1's remaining chunks; tile-0's pass-B softmax
            # is deferred into the middle of tile 1 to keep PE fed.
            et0 = epool.tile([128, D], BF16, tag="et")
            rp0 = small.tile([128, NCH], F32, tag="rp")

            def act_pair(ps, et, rp, ni):
                tt = small.tile([128, 512], BF16, tag="tt")
                nc.scalar.activation(tt[:], ps, AFT.Tanh, scale=tanh_scale)
                nc.scalar.activation(
                    et[:, ni * 512:(ni + 1) * 512], tt[:], AFT.Exp,
                    accum_out=rp[:, ni:ni + 1],
                )

            def p1_act(ps3, nis):
                for ni in nis:
                    j = ni % 3
                    act_pair(ps3[:, j * 512:(j + 1) * 512], et0, rp0, ni)

            ps3A = pA.tile([128, 3 * 512], F32)
            ps3B = pB.tile([128, 3 * 512], F32)
            if T > 1:
                et1 = epool.tile([128, D], BF16, tag="et")
                rp1 = small.tile([128, NCH], F32, tag="rp")
                ps_t1 = [psum_main.tile([128, 512], F32, name="ps")
                         for _ in range(2)]

            # the 8 bias matmuls depend only on wab — weave them into the
            # per-chunk arrival gaps of the Wa stream (one after each of
            # chunks 1..8) as PE filler
            def p1_bias(g):
                if g < 6:
                    ps3 = ps3A if g < 3 else ps3B
                    j = g % 3
                    nc.tensor.matmul(
                        ps3[:, j * 512:(j + 1) * 512],
                        atb0[:], wab[:, g * 512:(g + 1) * 512],
                        start=False, stop=False)
                elif T > 1:
                    ni = g - 6
                    nc.tensor.matmul(
                        ps_t1[ni][:], atb1[:],
                        wab[:, ni * 512:(ni + 1) * 512],
                        start=False, stop=False)

            for c in range(KCD):
                last_c = c == KCD - 1
                for half, ps3 in ((0, ps3A), (1, ps3B)):
                    for j in range(3):
                        ni = 3 * half + j
                        nc.tensor.matmul(
                            ps3[:, j * 512:(j + 1) * 512],
                            lhsT_of(at0, c), rhs_of(c, ni),
                            start=(c == 0), stop=last_c, perf_mode=pm)
                if T > 1:
                    for ni in range(2):
                        nc.tensor.matmul(
                            ps_t1[ni][:], lhsT_of(at1, c), rhs_of(c, ni),
                            start=(c == 0), stop=last_c, perf_mode=pm)
                if 4 <= c <= 11:
                    p1_bias(c - 4)
            def main_chunk(at, atb, et, rp, ni):
                ps = psum_main.tile([128, 512], F32, name="ps")
                mm_seq(ps[:], at, atb, ni, True, True)
                act_pair(ps[:], et, rp, ni)

            # Tile-0's six deferred softmax pairs are WOVEN between tile-1's
            # chunks on ScalarE: tile-1's pair must land in time to recycle
            # its PSUM slot, tile-0's pairs fill the gaps.
            if T > 1:
                for ni in range(2):
                    act_pair(ps_t1[ni][:], et1, rp1, ni)
                p1_act(ps3A, range(0, 1))
                main_chunk(at1, atb1, et1, rp1, 2)
                p1_act(ps3A, range(1, 2))
                main_chunk(at1, atb1, et1, rp1, 3)
                p1_act(ps3A, range(2, 3))
                es_a.close()
                main_chunk(at1, atb1, et1, rp1, 4)
                p1_act(ps3B, range(3, 4))
                main_chunk(at1, atb1, et1, rp1, 5)
                p1_act(ps3B, range(4, 6))
                es_b.close()
            else:
                p1_act(ps3A, range(0, 3))
                es_a.close()
                p1_act(ps3B, range(3, 6))
                es_b.close()

            # ---- phase 2: steady state; tile t-1's column-sum is emitted
            # after tile t's main matmuls so PE never waits on the softmax
            # reduction chain.
            with tc.tile_pool(name="psum_acc", bufs=1, space="PSUM") as psum_acc:
                psA = psum_acc.tile([NB, D], F32)

                def colsum(t, rp, et):
                    lhsT4 = softmax_tail(t, rp)
                    for ni in range(NCH):
                        nc.tensor.matmul(
                            psA[:, ni * 512:(ni + 1) * 512],
                            lhsT4[:],
                            et[:, ni * 512:(ni + 1) * 512],
                            start=(t == 0), stop=False,
                        )

                colsum(0, rp0, et0)
                prev = (1, rp1, et1) if T > 1 else None

                for t in range(2, T):
                    at = at_tile()
                    nc.sync.dma_start(at[:], at_h[t])
                    atb = at_pool.tile([128, 128], BF16, tag="atb",
                                       name="atb_sb")
                    nc.sync.dma_start(atb[:], atb_h[t])
                    et = epool.tile([128, D], BF16, tag="et")
                    rp = small.tile([128, NCH], F32, tag="rp")
                    if t < T - 1:
                        for ni in range(NCH):
                            main_chunk(at, atb, et, rp, ni)
                        colsum(*prev)
                        prev = (t, rp, et)
                    else:
                        # last tile: interleave the previous tile's column-sum
                        # and the +u matmuls between its chunks so PE has fill
                        # work while the final softmax chain resolves; the
                        # final column-sum then closes each psA group.
                        tp, rpp, etp = prev
                        lhsT4p = softmax_tail(tp, rpp)
                        for ni in range(NCH):
                            main_chunk(at, atb, et, rp, ni)
                            sl = slice(ni * 512, (ni + 1) * 512)
                            nc.tensor.matmul(
                                psA[:, sl], lhsT4p[:], etp[:, sl],
                                start=False, stop=False)
                        lhsT4 = softmax_tail(t, rp)
                        for ni in range(NCH):
                            sl = slice(ni * 512, (ni + 1) * 512)
                            nc.tensor.matmul(
                                psA[:, sl], lhsT4[:], et[:, sl],
                                start=False, stop=False)
                        prev = None
                if prev is not None:
                    colsum(*prev)
                # +u closes each psA group; the DVE multiplies pipeline
                # against the u-matmul stream
                for ni in range(NCH):
                    sl = slice(ni * 512, (ni + 1) * 512)
                    nc.tensor.matmul(
                        psA[:, sl], ub_sb[:], ones2[:],
                        start=False, stop=True)
                    outn = small.tile([NB, 512], F32)
                    nc.vector.tensor_mul(outn[:], psA[:, sl], trig_sb[:, sl])
                    nc.sync.dma_start(out_h[:, sl], outn[:])
            es_main.close()
    nc.compile()
    return nc


def kernel(h_state, x, trigger, mask, Wa, ba, Ws, bs, *, trace=False):
    global LAST_EXEC_NS
    h_state = np.asarray(h_state, dtype=np.float32)
    x = np.asarray(x, dtype=np.float32)
    trigger = np.asarray(trigger).astype(np.int64)
    mask = np.asarray(mask)
    Wa = np.asarray(Wa, dtype=np.float32)
    ba = np.asarray(ba, dtype=np.float32)
    Ws = np.asarray(Ws, dtype=np.float32)
    bs = np.asarray(bs, dtype=np.float32)
    fp8 = MODE == "fp8"

    # per-batch bias row (f64 for accuracy; dominates z's magnitude)
    s_sum = h_state.sum(axis=1, dtype=np.float64)                  # (B, 2048)
    bias = (s_sum @ Ws.astype(np.float64)
            + ba.astype(np.float64) + bs.astype(np.float64)).astype(np.float32)
    # bias rides in a bf16 chunk with one-hot value ALPHA; its PSUM
    # contribution must come out x(SC*SC) in fp8 mode (tanh rescales).
    zscale = SC * SC if fp8 else 1.0
    alpha = SC if fp8 else 1.0
    beta = zscale / alpha
    bias_hi = (bias * beta).astype(BF)
    bias_lo = (bias * beta - bias_hi.astype(np.float32)).astype(BF)  # (B, D)

    # trigger rows of a = [h_state | x]
    bi = np.arange(B)
    trig_full = np.concatenate(
        [h_state[bi, trigger], x[bi, trigger]], axis=1)            # (B, D)

    keep = [np.flatnonzero(np.asarray(mask[b]) != 0) for b in range(B)]
    n_rows_core = [
        sum(len(keep[c * NB + j]) for j in range(NB)) for c in range(NCORES)]
    T = max(1, max(math.ceil(r / 128) for r in n_rows_core))

    # shared quantized weight block
    if fp8:
        waq = np.clip(Wa * SC, -240.0, 240.0).astype(F8)
        # wa[c, p, r, n] = Wa_q[c*256 + r*128 + p, n]
        wa_dev = np.ascontiguousarray(
            waq.reshape(12, 2, 128, D).transpose(0, 2, 1, 3))
    else:
        wa_dev = np.ascontiguousarray(Wa.astype(BF).reshape(24, 128, D))

    in_maps = []
    for c in range(NCORES):
        rows_h = []           # compacted h_state rows
        rows_x = []           # compacted x rows
        owner = []            # batch-within-core per row
        for j in range(NB):
            b = c * NB + j
            idx = keep[b]
            rows_h.append(h_state[b, idx])
            rows_x.append(x[b, idx])
            owner.append(np.full(len(idx), j, dtype=np.int64))
        rows_h = np.concatenate(rows_h, axis=0)
        rows_x = np.concatenate(rows_x, axis=0)
        owner = np.concatenate(owner, axis=0)
        rc = rows_h.shape[0]
        r_idx = np.arange(rc)

        a_c = np.zeros((T * 128, D), dtype=np.float32)
        a_c[:rc, :KD] = rows_h
        a_c[:rc, KD:D] = rows_x
        if fp8:
            a_q = np.clip(a_c * SC, -240.0, 240.0).astype(F8)
            # at[t, p, c, r, m] = a_q[t*128+m, c*256 + r*128 + p]
            att = np.ascontiguousarray(
                a_q.reshape(T, 128, 12, 2, 128).transpose(0, 4, 2, 3, 1))
        else:
            att = np.ascontiguousarray(
                a_c.astype(BF).reshape(T, 128, 24, 128).transpose(0, 3, 2, 1))

        # bias chunk lhsT: atb[t, p, m] = alpha at p = 2*owner(+1) of row m
        atb = np.zeros((T * 128, 128), dtype=np.float32)
        atb[r_idx, 2 * owner] = alpha
        atb[r_idx, 2 * owner + 1] = alpha
        atb = np.ascontiguousarray(
            atb.astype(BF).reshape(T, 128, 128).transpose(0, 2, 1))

        # bias chunk rhs: rows 2j / 2j+1 = hi/lo of batch j
        wab = np.zeros((128, D), dtype=BF)
        for j in range(NB):
            b = c * NB + j
            wab[2 * j] = bias_hi[b]
            wab[2 * j + 1] = bias_lo[b]

        ind_all = np.zeros((128, T * NB), dtype=BF)
        ind_all[r_idx % 128, (r_idx // 128) * NB + owner] = 1.0

        trig = np.ascontiguousarray(trig_full[c * NB:(c + 1) * NB])
        u = np.array(
            [(S - len(keep[c * NB + j])) / np.float32(D) for j in range(NB)],
            dtype=np.float32)
        u_hi = u.astype(BF)
        u_lo = (u - u_hi.astype(np.float32)).astype(BF)
        ub = np.stack([u_hi, u_lo])                              # (2, NB)
        in_maps.append({"at": att, "atb": atb, "wa": wa_dev, "wab": wab,
                        "ind": ind_all, "trig": trig, "ub": ub})

    key = (T, MODE)
    if key not in _PROG_CACHE:
        _PROG_CACHE[key] = _build_program(T, MODE)
    nc = _PROG_CACHE[key]

    res = bass_utils.run_bass_kernel_spmd(
        nc, in_maps, list(range(NCORES)), trace=trace)
    LAST_EXEC_NS = res.exec_time_ns
    return np.concatenate(
        [np.asarray(res.results[c]["out"]) for c in range(NCORES)], axis=0)
